# revision 35
# baseline (speedup 1.0000x reference)
"""Bayer mosaic channel selection on 8 Trainium2 NeuronCores.

Reference computes out[b, i, j] = img[b, c(i,j), i, j] with
    c = 1 where (i+j) even
    c = 2 where i even and j odd
    c = 0 where i odd and j even
so each output row interleaves two channels at element granularity:
    even rows:  ch1 @ even cols, ch2 @ odd cols
    odd rows:   ch0 @ even cols, ch1 @ odd cols

Sharding: pure data-parallel, one batch image per NeuronCore (B == 8).

Per-core plan (rows processed in blocks of 256 = 128 even + 128 odd):
  - Two strided-row 2 MiB DMA loads per block pull only the rows that are
    used (ch1 all rows, ch2 even rows, ch0 odd rows -> 2/3 of the input),
    de-interleaving row parities so each SBUF partition holds one output
    row. Each load pairs chunks (H*W - W) elements apart; pairing them
    exactly H*W apart (16 MiB, power-of-two aligned) measurably drops HBM
    throughput from ~416 to ~347 GB/s (bank aliasing).
  - Four stride-2 vector-engine copies assemble the output rows in a
    separate SBUF area (each output element is touched exactly once).
  - Two 1 MiB DMA half-stores per block (even rows after the first two
    copies, odd rows after the last two) write back with row parities
    re-interleaved; the fine store granularity keeps the SDMA engines fed
    through the pipeline tail.
All DMA traffic moves in 8 KiB contiguous chunks; measured 120 us of DMA
busy time for 50.3 MiB/core (~416 GB/s, the 16-SDMA-engine line rate)
inside a 128 us kernel -- the remainder is the fixed NEFF prologue.

Written in raw Bass (not Tile): walrus codegen caps packed sync-wait
conditions at two per instruction, which Tile's auto-generated semaphores
exceed for this dependency pattern; raw streams emit waits as standalone
instructions. One semaphore per buffer slot with explicit acknowledge
waits keeps multi-DMA completion counting unambiguous (CoreSim race
detector verified).
"""

from contextlib import ExitStack

import numpy as np

import concourse.bass as bass
import concourse.mybir as mybir

B, C, H, W = 8, 3, 2048, 2048
P = 128
RB = 2 * P
NBLK = H // RB
NBUF = 3

_NC_CACHE: list = []


def build_nc():
    f32 = mybir.dt.float32
    nc = bass.Bass()
    img = nc.declare_dram_parameter("img", [C, H, W], f32, isOutput=False)
    out = nc.declare_dram_parameter("out", [H, W], f32, isOutput=True)

    out_r = out.rearrange("(p two) w -> p two w", two=2)

    with ExitStack() as ctx:
        ctx.enter_context(nc.cleanup_on_exit())
        # tin: seg0 = ch0 @ odd rows, seg1 = ch1 @ even rows,
        #      seg2 = ch1 @ odd rows, seg3 = ch2 @ even rows
        tin = [
            ctx.enter_context(nc.sbuf_tensor(f"tin{i}", [P, 4 * W], f32))
            for i in range(NBUF)
        ]
        ob = [
            ctx.enter_context(nc.sbuf_tensor(f"ob{i}", [P, 2 * W], f32))
            for i in range(NBUF)
        ]
        sem_in = [
            ctx.enter_context(nc.semaphore(f"sem_in{i}")) for i in range(NBUF)
        ]
        sem_st = [
            ctx.enter_context(nc.semaphore(f"sem_st{i}")) for i in range(NBUF)
        ]
        sem_cp = ctx.enter_context(nc.semaphore("sem_cp"))

        with nc.Block() as block:

            @block.sync
            def _(sync):
                for k in range(NBLK):
                    j, r = k % NBUF, k // NBUF
                    if k >= NBUF:
                        sync.wait_ge(sem_cp, 4 * (k - NBUF) + 4)
                        sync.wait_ge(sem_in[j], 32 * r)
                    base = k * RB * W + W
                    src_a = bass.AP(
                        img, base, [[2 * W, P], [H * W - W, 2], [1, W]]
                    )
                    src_b = bass.AP(
                        img, base + H * W, [[2 * W, P], [H * W - W, 2], [1, W]]
                    )
                    sync.dma_start(
                        out=tin[j][:, 0 : 2 * W].rearrange("p (s w) -> p s w", w=W),
                        in_=src_a,
                    ).then_inc(sem_in[j], 16)
                    sync.dma_start(
                        out=tin[j][:, 2 * W : 4 * W].rearrange("p (s w) -> p s w", w=W),
                        in_=src_b,
                    ).then_inc(sem_in[j], 16)

            @block.vector
            def _(vector):
                for k in range(NBLK):
                    j, r = k % NBUF, k // NBUF
                    vector.wait_ge(sem_in[j], 32 * (r + 1))
                    if k >= NBUF:
                        vector.wait_ge(sem_st[j], 32 * r)
                    t, o = tin[j], ob[j]
                    # even rows: ch1 @ even cols (seg1), ch2 @ odd cols (seg3)
                    vector.tensor_copy(
                        o[:, 0:W:2], t[:, W : 2 * W : 2]
                    ).then_inc(sem_cp, 1)
                    vector.tensor_copy(
                        o[:, 1:W:2], t[:, 3 * W + 1 : 4 * W : 2]
                    ).then_inc(sem_cp, 1)
                    # odd rows: ch0 @ even cols (seg0), ch1 @ odd cols (seg2)
                    vector.tensor_copy(
                        o[:, W : 2 * W : 2], t[:, 0:W:2]
                    ).then_inc(sem_cp, 1)
                    vector.tensor_copy(
                        o[:, W + 1 : 2 * W : 2], t[:, 2 * W + 1 : 3 * W : 2]
                    ).then_inc(sem_cp, 1)

            @block.scalar
            def _(scalar):
                for k in range(NBLK):
                    j, r = k % NBUF, k // NBUF
                    pr0 = k * P
                    scalar.wait_ge(sem_cp, 4 * k + 2)
                    if k >= NBUF:
                        scalar.wait_ge(sem_st[j], 32 * r)
                    scalar.dma_start(
                        out=out_r[pr0 : pr0 + P, 0, :],
                        in_=ob[j][:, 0:W],
                    ).then_inc(sem_st[j], 16)
                    scalar.wait_ge(sem_cp, 4 * k + 4)
                    scalar.dma_start(
                        out=out_r[pr0 : pr0 + P, 1, :],
                        in_=ob[j][:, W : 2 * W],
                    ).then_inc(sem_st[j], 16)

    return nc


def _get_nc():
    if not _NC_CACHE:
        _NC_CACHE.append(build_nc())
    return _NC_CACHE[0]


def kernel(**inputs) -> np.ndarray:
    img = np.asarray(inputs["img"], dtype=np.float32)
    assert img.shape == (B, C, H, W), img.shape

    from concourse.bass_utils import run_bass_kernel_spmd

    nc = _get_nc()
    in_maps = [{"img": np.ascontiguousarray(img[b])} for b in range(B)]
    res = run_bass_kernel_spmd(nc, in_maps, core_ids=list(range(B)))
    return np.stack([res.results[i]["out"] for i in range(B)], axis=0)
